# revision 14
# baseline (speedup 1.0000x reference)
"""ChebConv2D (K1=K2=3) Trainium2 Bass kernel.

Data-parallel over batch (B=8) across 8 NeuronCores; per core the whole
per-batch computation runs on-chip.

Math (per batch, x: [N, N, C], N=200, C=32, OUT=64):
    out = U_0 + R_L(U_1) + R_{L^2}(U_2) + bias
    U_j = sum_i (A^i x) @ W'_ij^T      (Chebyshev folded into W' on host)

v5: host supplies xt = x^T directly (TT rows 0:32), so S1 only computes
the L/L2 powers (N=400 streams, one psum tile). hop1 split across
sync/scalar HWDGE, hop2 pipelined in 10 pieces on the gpsimd SWDGE ring,
out stored as [n2, n1, o] fp16 on gpsimd, drains balanced DVE/ACT.
"""

import numpy as np

import concourse.bass as bass
import concourse.mybir as mybir
from concourse import bacc
import concourse.tile as tile
from concourse import bass_utils


N = 200
C = 32
OUT = 64
B = 8
NC_HALF = 100
BLK = 8
NBLK = N // BLK
F32 = mybir.dt.float32
F16 = mybir.dt.float16
MIXN = 192
XCH = 10


def build_program():
    nc = bacc.Bacc("TRN2")

    x_d = nc.dram_tensor("x", [N, N * C], F16, kind="ExternalInput")
    xt_d = nc.dram_tensor("xt", [C, N * N], F16, kind="ExternalInput")
    g_d = nc.dram_tensor("g", [N, 2 * N], F16, kind="ExternalInput")
    ws_d = nc.dram_tensor("ws", [C * 3 + 1, MIXN], F16, kind="ExternalInput")
    lt1_d = nc.dram_tensor("lt1", [N, N], F16, kind="ExternalInput")
    lt2_d = nc.dram_tensor("lt2", [N, N], F16, kind="ExternalInput")
    ones_d = nc.dram_tensor("ones", [1, N * N], F16, kind="ExternalInput")
    # out layout [n2, n1, o] (host transposes back)
    out_d = nc.dram_tensor("out", [N, N, OUT], F16, kind="ExternalOutput")
    # transpose scratch [c][n2][i][n1], i in {L, L2}
    scr_d = nc.dram_tensor("scr", [C, N, 2, N], F16, kind="Internal")

    with tile.TileContext(nc) as tc:
        with (
            tc.tile_pool(name="const", bufs=1) as constp,
            tc.tile_pool(name="tt", bufs=1) as ttp,
            tc.tile_pool(name="u0", bufs=1) as u0p,
        ):
            g_t = []
            lt_t = {}
            for t in range(2):
                g = constp.tile([NC_HALF, 2 * N], F16, tag=f"g{t}")
                nc.scalar.dma_start(g[:], g_d[t * NC_HALF:(t + 1) * NC_HALF, :])
                g_t.append(g)
                for j in (1, 2):
                    lt = constp.tile([NC_HALF, N], F16, tag=f"lt{j}{t}")
                    src = lt1_d if j == 1 else lt2_d
                    nc.scalar.dma_start(lt[:], src[t * NC_HALF:(t + 1) * NC_HALF, :])
                    lt_t[(j, t)] = lt
            ws = constp.tile([C * 3 + 1, MIXN], F16, tag="ws")
            nc.scalar.dma_start(ws[:], ws_d[:, :])

            TT = ttp.tile([C * 3 + 1, N * N], F16, tag="TT")
            nc.sync.dma_start(TT[96:97, :], ones_d[:, :])
            TT3 = TT[:].rearrange("p (a b) -> p a b", b=N)

            # U half 0 for all n1: [n2 0..99, n1*192 + (j,o)]
            UC0 = u0p.tile([NC_HALF, N * MIXN], F16, tag="UC0")
            UC03 = UC0[:].rearrange("p (n f) -> p n f", f=MIXN)

            with (
                tc.tile_pool(name="xa", bufs=3) as xap,
                tc.tile_pool(name="sg", bufs=16) as sgp,
                tc.tile_pool(name="uc", bufs=3) as ucp,
                tc.tile_pool(name="ob", bufs=2) as obp,
                tc.tile_pool(name="psU", bufs=3, space="PSUM") as psup,
            ):
                psap_cm = tc.tile_pool(name="psA", bufs=3, space="PSUM")
                psap = psap_cm.__enter__()

                xq = {}
                drain_flip = [0]

                def x_load(mg):
                    for t in range(2):
                        xm = xap.tile([NC_HALF, XCH * 128], F16,
                                      tag=f"xm{t}", name=f"xm{t}_{mg}")
                        nc.gpsimd.dma_start(
                            xm[:], x_d[t * NC_HALF:(t + 1) * NC_HALF,
                                       mg * 128:(mg + XCH) * 128])
                        xq[(t, mg)] = xm

                def s1_chunk(m):
                    mg = (m // XCH) * XCH
                    ng = mg + 2 * XCH
                    if m % XCH == 0 and ng < 50 and (0, ng) not in xq:
                        x_load(ng)
                    mm = m % XCH
                    psg = psap.tile([128, 400], F32, tag="psg",
                                    name=f"psg_{m}")
                    for t in range(2):
                        lhsT = xq[(t, mg)][:, mm * 128:(mm + 1) * 128]
                        nc.tensor.matmul(psg[:], lhsT, g_t[t][:, :],
                                         start=(t == 0), stop=(t == 1))
                    sc = sgp.tile([128, 400], F16, tag="sc", name=f"sc_{m}")
                    nc.vector.tensor_copy(sc[:], psg[:])
                    # hop1: one DMA -> scratch [c][n2][i][n1]
                    dst = scr_d[:, 4 * m:4 * m + 4, :, :]
                    dst = dst.rearrange("d r i b -> r d (i b)")
                    eng = nc.sync if m % 2 == 0 else nc.scalar
                    eng.dma_start(dst, sc[:, :])

                def hop2_piece(gp):
                    # 40 n2 rows per piece, one DMA per power
                    for i in range(2):
                        src = scr_d[:, gp * 40:(gp + 1) * 40, i, :]
                        dst = TT3[(i + 1) * 32:(i + 2) * 32,
                                  gp * 40:(gp + 1) * 40, :]
                        nc.gpsimd.dma_start(dst, src)

                def s2_pair(p2, h, dst3, slot, dve_w=2):
                    psu = psup.tile([NC_HALF, 2 * MIXN], F32, tag="psu",
                                    name=f"psu{h}_{p2}")
                    for q in range(2):
                        n1 = p2 * 2 + q
                        lhsT = TT3[0:97, h * NC_HALF:(h + 1) * NC_HALF,
                                   n1:n1 + 1]
                        nc.tensor.matmul(psu[:, q * MIXN:(q + 1) * MIXN],
                                         lhsT, ws[:], start=True, stop=True)
                    dst = dst3[:, slot * 2:slot * 2 + 2, :]
                    psu3 = psu[:].rearrange("p (q f) -> p q f", f=MIXN)
                    if drain_flip[0] % 4 < dve_w:
                        nc.vector.tensor_copy(dst, psu3)
                    else:
                        nc.scalar.copy(dst, psu3)
                    drain_flip[0] += 1

                # ---- phase 1: S1 chunks 0..29 + hop2 pieces ----
                x_load(0)
                x_load(10)
                nc.gpsimd.dma_start(TT[0:32, :], xt_d[:, :])
                for m in range(30):
                    s1_chunk(m)
                    if m % 10 == 9:
                        hop2_piece(m // 10)

                # ---- phase 2: S1 chunks 30..49 + S2 h=0 pairs ----
                p2done = 0
                for k in range(20):
                    m = 30 + k
                    s1_chunk(m)
                    if m % 10 == 9:
                        hop2_piece(m // 10)
                    if k >= 8:
                        for _ in range(9):
                            if p2done < 100:
                                s2_pair(p2done, 0, UC03, p2done)
                                p2done += 1
                while p2done < 100:
                    s2_pair(p2done, 0, UC03, p2done)
                    p2done += 1
                psap_cm.__exit__(None, None, None)

                # ---- phase 3: S2 h=1 + S3 per block ----
                psop_cm = tc.tile_pool(name="psO", bufs=2, space="PSUM")
                psop = psop_cm.__enter__()
                ob = None
                for blk in range(NBLK):
                    uc1 = ucp.tile([NC_HALF, BLK * MIXN], F16, tag="uc1",
                                   name=f"uc1_{blk}")
                    uc13 = uc1[:].rearrange("p (n f) -> p n f", f=MIXN)
                    for q in range(4):
                        s2_pair(blk * 4 + q, 1, uc13, q, dve_w=1)
                    b2 = blk % 2
                    if b2 == 0:
                        ob = obp.tile([NC_HALF, 4 * 512], F16, tag="ob",
                                      name=f"ob_{blk}")
                    for m2 in range(2):
                        pso = psop.tile([NC_HALF, BLK * OUT], F32, tag="pso",
                                        name=f"pso_{blk}_{m2}")
                        k = 0
                        for j in (1, 2):
                            for h in range(2):
                                lhsT = lt_t[(j, h)][:,
                                                    m2 * NC_HALF:(m2 + 1) * NC_HALF]
                                if h == 0:
                                    rhs = UC03[:, blk * BLK:(blk + 1) * BLK,
                                               j * OUT:(j + 1) * OUT]
                                else:
                                    rhs = uc13[:, :, j * OUT:(j + 1) * OUT]
                                nc.tensor.matmul(pso[:], lhsT, rhs,
                                                 start=(k == 0), stop=(k == 3))
                                k += 1
                        pso3 = pso[:].rearrange("p (n o) -> p n o", o=OUT)
                        if m2 == 0:
                            u0 = UC03[:, blk * BLK:(blk + 1) * BLK, 0:OUT]
                        else:
                            u0 = uc13[:, :, 0:OUT]
                        off = (m2 * 2 + b2) * 512
                        dst = ob[:, off:off + 512].rearrange(
                            "p (n o) -> p n o", o=OUT)
                        nc.vector.tensor_add(dst, pso3, u0)
                    if b2 == 1:
                        src = ob[:].rearrange("p (m f) -> p m f", m=2)
                        n1lo = (blk - 1) * BLK
                        dst = out_d[:, n1lo:n1lo + 2 * BLK, :].rearrange(
                            "(m p) n o -> p m (n o)", m=2)
                        nc.gpsimd.dma_start(dst, src)
                    elif blk == NBLK - 1:
                        src = ob[:].rearrange("p (m f) -> p m f", m=2)[:, :, 0:512]
                        dst = out_d[:, blk * BLK:(blk + 1) * BLK, :].rearrange(
                            "(m p) n o -> p m (n o)", m=2)
                        nc.gpsimd.dma_start(dst, src)
                psop_cm.__exit__(None, None, None)
    nc.compile()
    return nc


def _host_inputs(adj, weight, bias):
    adj = np.asarray(adj, np.float64)
    weight = np.asarray(weight, np.float64)
    bias = np.asarray(bias, np.float64)
    n = adj.shape[0]
    A = adj * (1.0 - np.eye(n))
    d0 = A.sum(0) ** -0.5
    d1 = A.sum(1) ** -0.5
    d0[np.isinf(d0)] = 0.0
    d1[np.isinf(d1)] = 0.0
    L = d0[:, None] * A * d1[None, :]
    L2 = L @ L

    p = np.array([[1.0, 0, 0], [0, 1.0, 0], [-1.0, 0, 2.0]])
    W = weight.reshape(OUT, 3, 3, C)
    Wp = np.einsum("ai,bj,oabc->ijoc", p, p, W)

    G = np.concatenate([L, L2], axis=1)
    WS = np.zeros((3 * C + 1, MIXN))
    for i in range(3):
        for j in range(3):
            WS[i * C:(i + 1) * C, j * OUT:(j + 1) * OUT] = Wp[i, j].T
    WS[96, 0:OUT] = bias
    ones = np.ones((1, n * n))
    return (G.astype(np.float16), WS.astype(np.float16),
            np.ascontiguousarray(L.T).astype(np.float16),
            np.ascontiguousarray(L2.T).astype(np.float16),
            ones.astype(np.float16))


_PROGRAM = None


def _in_maps(x, adj, weight, bias):
    G, WS, LT1, LT2, ONES = _host_inputs(adj, weight, bias)
    x = np.asarray(x)
    x16 = x.astype(np.float16)
    maps = []
    for b in range(B):
        maps.append({
            "x": np.ascontiguousarray(x16[b].reshape(N, N * C)),
            "xt": np.ascontiguousarray(
                x16[b].transpose(2, 1, 0).reshape(C, N * N)),
            "g": G, "ws": WS, "lt1": LT1, "lt2": LT2, "ones": ONES,
        })
    return maps


def _postprocess(res):
    # out stored as [n2, n1, o] fp16 -> [n1, n2, o] fp32
    return np.stack(
        [np.asarray(res.results[b]["out"]).transpose(1, 0, 2)
         for b in range(B)], axis=0).astype(np.float32)


def kernel(x, adj, weight, bias):
    global _PROGRAM
    if _PROGRAM is None:
        _PROGRAM = build_program()
    res = bass_utils.run_bass_kernel_spmd(
        _PROGRAM, _in_maps(x, adj, weight, bias), core_ids=list(range(B)))
    return _postprocess(res)


# revision 19
# speedup vs baseline: 1.0375x; 1.0375x over previous
"""ChebConv2D (K1=K2=3) Trainium2 Bass kernel.

Data-parallel over batch (B=8) across 8 NeuronCores; per core the whole
per-batch computation runs on-chip.

Math (per batch, x: [N, N, C], N=200, C=32, OUT=64):
    out = U_0 + R_L(U_1) + R_{L^2}(U_2) + bias
    U_j = sum_i (A^i x) @ W'_ij^T      (Chebyshev folded into W' on host)

v5: host supplies xt = x^T directly (TT rows 0:32), so S1 only computes
the L/L2 powers (N=400 streams, one psum tile). hop1 split across
sync/scalar HWDGE, hop2 pipelined in 10 pieces on the gpsimd SWDGE ring,
out stored as [n2, n1, o] fp16 on gpsimd, drains balanced DVE/ACT.
"""

import numpy as np

import concourse.bass as bass
import concourse.mybir as mybir
from concourse import bacc
import concourse.tile as tile
from concourse import bass_utils


N = 200
C = 32
OUT = 64
B = 8
NC_HALF = 100
BLK = 8
NBLK = N // BLK
F32 = mybir.dt.float32
F16 = mybir.dt.float16
MIXN = 192
XCH = 10


def build_program():
    nc = bacc.Bacc("TRN2")

    x_d = nc.dram_tensor("x", [N, N * C], F16, kind="ExternalInput")
    xt_d = nc.dram_tensor("xt", [C, N * N], F16, kind="ExternalInput")
    g_d = nc.dram_tensor("g", [N, 2 * N], F16, kind="ExternalInput")
    ws_d = nc.dram_tensor("ws", [C * 3 + 1, MIXN], F16, kind="ExternalInput")
    lt1_d = nc.dram_tensor("lt1", [N, N], F16, kind="ExternalInput")
    lt2_d = nc.dram_tensor("lt2", [N, N], F16, kind="ExternalInput")
    ones_d = nc.dram_tensor("ones", [1, N * N], F16, kind="ExternalInput")
    # out layout [n2, n1, o] (host transposes back)
    out_d = nc.dram_tensor("out", [N, N, OUT], F16, kind="ExternalOutput")
    # transpose scratch [c][n2][i][n1], i in {L, L2}
    scr_d = nc.dram_tensor("scr", [C, N, 2, N], F16, kind="Internal")

    with tile.TileContext(nc) as tc:
        with (
            tc.tile_pool(name="const", bufs=1) as constp,
            tc.tile_pool(name="tt", bufs=1) as ttp,
            tc.tile_pool(name="u0", bufs=1) as u0p,
        ):
            g_t = []
            lt_t = {}
            for t in range(2):
                g = constp.tile([NC_HALF, 2 * N], F16, tag=f"g{t}")
                nc.scalar.dma_start(g[:], g_d[t * NC_HALF:(t + 1) * NC_HALF, :])
                g_t.append(g)
                for j in (1, 2):
                    lt = constp.tile([NC_HALF, N], F16, tag=f"lt{j}{t}")
                    src = lt1_d if j == 1 else lt2_d
                    nc.scalar.dma_start(lt[:], src[t * NC_HALF:(t + 1) * NC_HALF, :])
                    lt_t[(j, t)] = lt
            ws = constp.tile([C * 3 + 1, MIXN], F16, tag="ws")
            nc.scalar.dma_start(ws[:], ws_d[:, :])

            TT = ttp.tile([C * 3 + 1, N * N], F16, tag="TT")
            nc.sync.dma_start(TT[96:97, :], ones_d[:, :])
            TT3 = TT[:].rearrange("p (a b) -> p a b", b=N)

            # U half 0 for all n1: [n2 0..99, n1*192 + (j,o)]
            UC0 = u0p.tile([NC_HALF, N * MIXN], F16, tag="UC0")
            UC03 = UC0[:].rearrange("p (n f) -> p n f", f=MIXN)

            with (
                tc.tile_pool(name="xa", bufs=4) as xap,
                tc.tile_pool(name="sg", bufs=14) as sgp,
                tc.tile_pool(name="uc", bufs=3) as ucp,
                tc.tile_pool(name="ob", bufs=2) as obp,
                tc.tile_pool(name="psU", bufs=3, space="PSUM") as psup,
            ):
                psap_cm = tc.tile_pool(name="psA", bufs=3, space="PSUM")
                psap = psap_cm.__enter__()

                xq = {}
                drain_flip = [0]

                def x_load(mg):
                    for t in range(2):
                        xm = xap.tile([NC_HALF, XCH * 128], F16,
                                      tag=f"xm{t}", name=f"xm{t}_{mg}")
                        nc.gpsimd.dma_start(
                            xm[:], x_d[t * NC_HALF:(t + 1) * NC_HALF,
                                       mg * 128:(mg + XCH) * 128])
                        xq[(t, mg)] = xm

                def s1_chunk(m):
                    mg = (m // XCH) * XCH
                    mm = m % XCH
                    psg = psap.tile([128, 400], F32, tag="psg",
                                    name=f"psg_{m}")
                    for t in range(2):
                        lhsT = xq[(t, mg)][:, mm * 128:(mm + 1) * 128]
                        nc.tensor.matmul(psg[:], lhsT, g_t[t][:, :],
                                         start=(t == 0), stop=(t == 1))
                    sc = sgp.tile([128, 400], F16, tag="sc", name=f"sc_{m}")
                    if m % 2 == 0:
                        nc.vector.tensor_copy(sc[:], psg[:])
                    else:
                        nc.scalar.copy(sc[:], psg[:])
                    # hop1: one DMA -> scratch [c][n2][i][n1]
                    dst = scr_d[:, 4 * m:4 * m + 4, :, :]
                    dst = dst.rearrange("d r i b -> r d (i b)")
                    eng = nc.sync if m % 2 == 0 else nc.scalar
                    eng.dma_start(dst, sc[:, :])

                def hop2_piece(gp):
                    # 40 n2 rows per piece, one DMA per power
                    for i in range(2):
                        src = scr_d[:, gp * 40:(gp + 1) * 40, i, :]
                        dst = TT3[(i + 1) * 32:(i + 2) * 32,
                                  gp * 40:(gp + 1) * 40, :]
                        nc.gpsimd.dma_start(dst, src)

                def s2_pair(p2, h, dst3, slot, dve_w=2):
                    psu = psup.tile([NC_HALF, 2 * MIXN], F32, tag="psu",
                                    name=f"psu{h}_{p2}")
                    for q in range(2):
                        n1 = p2 * 2 + q
                        lhsT = TT3[0:97, h * NC_HALF:(h + 1) * NC_HALF,
                                   n1:n1 + 1]
                        nc.tensor.matmul(psu[:, q * MIXN:(q + 1) * MIXN],
                                         lhsT, ws[:], start=True, stop=True)
                    dst = dst3[:, slot * 2:slot * 2 + 2, :]
                    psu3 = psu[:].rearrange("p (q f) -> p q f", f=MIXN)
                    if drain_flip[0] % 4 < dve_w:
                        nc.vector.tensor_copy(dst, psu3)
                    else:
                        nc.scalar.copy(dst, psu3)
                    drain_flip[0] += 1

                # ---- phase 1: S1 chunks 0..29 + hop2 pieces ----
                for mg in range(0, 50, XCH):
                    x_load(mg)
                xt3 = TT3[0:32, :, :]
                for m in range(30):
                    s1_chunk(m)
                    if m % 3 == 2 and m // 3 < 10:
                        p = m // 3
                        nc.gpsimd.dma_start(
                            xt3[:, p * 20:(p + 1) * 20, :],
                            xt_d[:, p * 20 * N:(p + 1) * 20 * N])
                    if m % 10 == 9:
                        hop2_piece(m // 10)

                # ---- phase 2: S1 chunks 30..49 + S2 h=0 pairs ----
                p2done = 0
                for k in range(20):
                    m = 30 + k
                    s1_chunk(m)
                    if m % 10 == 9:
                        hop2_piece(m // 10)
                    if k >= 8:
                        for _ in range(9):
                            if p2done < 100:
                                s2_pair(p2done, 0, UC03, p2done)
                                p2done += 1
                while p2done < 100:
                    s2_pair(p2done, 0, UC03, p2done)
                    p2done += 1
                psap_cm.__exit__(None, None, None)

                # ---- phase 3: S2 h=1 + S3 per block ----
                psop_cm = tc.tile_pool(name="psO", bufs=2, space="PSUM")
                psop = psop_cm.__enter__()
                ob = None
                for blk in range(NBLK):
                    uc1 = ucp.tile([NC_HALF, BLK * MIXN], F16, tag="uc1",
                                   name=f"uc1_{blk}")
                    uc13 = uc1[:].rearrange("p (n f) -> p n f", f=MIXN)
                    for q in range(4):
                        s2_pair(blk * 4 + q, 1, uc13, q, dve_w=1)
                    b2 = blk % 2
                    if b2 == 0:
                        ob = obp.tile([NC_HALF, 4 * 512], F16, tag="ob",
                                      name=f"ob_{blk}")
                    for m2 in range(2):
                        pso = psop.tile([NC_HALF, BLK * OUT], F32, tag="pso",
                                        name=f"pso_{blk}_{m2}")
                        k = 0
                        for j in (1, 2):
                            for h in range(2):
                                lhsT = lt_t[(j, h)][:,
                                                    m2 * NC_HALF:(m2 + 1) * NC_HALF]
                                if h == 0:
                                    rhs = UC03[:, blk * BLK:(blk + 1) * BLK,
                                               j * OUT:(j + 1) * OUT]
                                else:
                                    rhs = uc13[:, :, j * OUT:(j + 1) * OUT]
                                nc.tensor.matmul(pso[:], lhsT, rhs,
                                                 start=(k == 0), stop=(k == 3))
                                k += 1
                        pso3 = pso[:].rearrange("p (n o) -> p n o", o=OUT)
                        if m2 == 0:
                            u0 = UC03[:, blk * BLK:(blk + 1) * BLK, 0:OUT]
                        else:
                            u0 = uc13[:, :, 0:OUT]
                        off = (m2 * 2 + b2) * 512
                        dst = ob[:, off:off + 512].rearrange(
                            "p (n o) -> p n o", o=OUT)
                        nc.vector.tensor_add(dst, pso3, u0)
                    if b2 == 1:
                        src = ob[:].rearrange("p (m f) -> p m f", m=2)
                        n1lo = (blk - 1) * BLK
                        dst = out_d[:, n1lo:n1lo + 2 * BLK, :].rearrange(
                            "(m p) n o -> p m (n o)", m=2)
                        nc.gpsimd.dma_start(dst, src)
                    elif blk == NBLK - 1:
                        src = ob[:].rearrange("p (m f) -> p m f", m=2)[:, :, 0:512]
                        dst = out_d[:, blk * BLK:(blk + 1) * BLK, :].rearrange(
                            "(m p) n o -> p m (n o)", m=2)
                        nc.gpsimd.dma_start(dst, src)
                psop_cm.__exit__(None, None, None)
    nc.compile()
    return nc


def _host_inputs(adj, weight, bias):
    adj = np.asarray(adj, np.float64)
    weight = np.asarray(weight, np.float64)
    bias = np.asarray(bias, np.float64)
    n = adj.shape[0]
    A = adj * (1.0 - np.eye(n))
    d0 = A.sum(0) ** -0.5
    d1 = A.sum(1) ** -0.5
    d0[np.isinf(d0)] = 0.0
    d1[np.isinf(d1)] = 0.0
    L = d0[:, None] * A * d1[None, :]
    L2 = L @ L

    p = np.array([[1.0, 0, 0], [0, 1.0, 0], [-1.0, 0, 2.0]])
    W = weight.reshape(OUT, 3, 3, C)
    Wp = np.einsum("ai,bj,oabc->ijoc", p, p, W)

    G = np.concatenate([L, L2], axis=1)
    WS = np.zeros((3 * C + 1, MIXN))
    for i in range(3):
        for j in range(3):
            WS[i * C:(i + 1) * C, j * OUT:(j + 1) * OUT] = Wp[i, j].T
    WS[96, 0:OUT] = bias
    ones = np.ones((1, n * n))
    return (G.astype(np.float16), WS.astype(np.float16),
            np.ascontiguousarray(L.T).astype(np.float16),
            np.ascontiguousarray(L2.T).astype(np.float16),
            ones.astype(np.float16))


_PROGRAM = None


def _in_maps(x, adj, weight, bias):
    G, WS, LT1, LT2, ONES = _host_inputs(adj, weight, bias)
    x = np.asarray(x)
    x16 = x.astype(np.float16)
    maps = []
    for b in range(B):
        maps.append({
            "x": np.ascontiguousarray(x16[b].reshape(N, N * C)),
            "xt": np.ascontiguousarray(
                x16[b].transpose(2, 1, 0).reshape(C, N * N)),
            "g": G, "ws": WS, "lt1": LT1, "lt2": LT2, "ones": ONES,
        })
    return maps


def _postprocess(res):
    # out stored as [n2, n1, o] fp16 -> [n1, n2, o] fp32
    return np.stack(
        [np.asarray(res.results[b]["out"]).transpose(1, 0, 2)
         for b in range(B)], axis=0).astype(np.float32)


def kernel(x, adj, weight, bias):
    global _PROGRAM
    if _PROGRAM is None:
        _PROGRAM = build_program()
    res = bass_utils.run_bass_kernel_spmd(
        _PROGRAM, _in_maps(x, adj, weight, bias), core_ids=list(range(B)))
    return _postprocess(res)
